# revision 5
# baseline (speedup 1.0000x reference)
"""BRPConvEmbedding (3-layer GraphConv + AvgPool readout) on 8 Trainium2 cores.

Sharding: graphs are split contiguously across cores (32 graphs/core), so
pooling is core-local and the output is a pure concat. Each core owns the
nodes of its graphs; within a core, nodes are permuted into dst-groups of 64
nodes whose total in-degree per src-half is capped at 512 (4 chunks of 128
edge slots) via greedy bin-packing, which makes the per-group edge-chunk
layout uniform across all cores (single SPMD program).

Per layer: hn rows are fetched with SWDGE dma_gather (int16 indices; the node
table is split into two halves so indices fit in int16), the per-edge one-hot
is built on the VectorE (iota + tensor_tensor is_equal), the segment-sum runs
on the TensorE (lhsT=gathered chunk, rhs=onehot, PSUM accumulation), followed
by agg.T @ W + fused epilogue, and an AllGather of the new node features.
"""
import numpy as np
from contextlib import ExitStack

import concourse.bacc as bacc
import concourse.mybir as mybir
from concourse import tile
from concourse.bass_utils import run_bass_kernel_spmd

N_NODES = 50000
N_EDGES = 800000
D = 128
N_LAYERS = 3
N_GRAPHS = 256
NCORES = 8
GSZ = 64                  # dst nodes per group
CHUNKS_PER_HALF = 4       # 4*128 = 512 edge-slot cap per (group, half)
CAP = CHUNKS_PER_HALF * 128
GPC = N_GRAPHS // NCORES  # graphs per core


# ----------------------------------------------------------------- host prep
def _pack_groups(nodes, dA, dB):
    """Greedy bin-packing of nodes into groups of <= GSZ nodes with
    sum(dA) <= CAP and sum(dB) <= CAP per group. Returns group id per node."""
    order = np.argsort(-(dA + dB), kind="stable")
    gids = np.full(len(nodes), -1, dtype=np.int64)
    usedA, usedB, usedN = [], [], []
    for i in order:
        a, b = dA[i], dB[i]
        best, best_load = -1, 2.0
        for g in range(len(usedA)):
            if usedN[g] < GSZ and usedA[g] + a <= CAP and usedB[g] + b <= CAP:
                load = max((usedA[g] + a) / CAP, (usedB[g] + b) / CAP)
                if load < best_load:
                    best, best_load = g, load
        if best < 0:
            usedA.append(0), usedB.append(0), usedN.append(0)
            best = len(usedA) - 1
        gids[i] = best
        usedA[best] += a
        usedB[best] += b
        usedN[best] += 1
    return gids, len(usedA)


def preprocess(feats, W, b, src, dst, graph_ids):
    src = np.asarray(src).astype(np.int64)
    dst = np.asarray(dst).astype(np.int64)
    graph_ids = np.asarray(graph_ids).astype(np.int64)
    feats = np.asarray(feats, dtype=np.float32)

    deg_out = np.maximum(np.bincount(src, minlength=N_NODES), 1).astype(np.float32)
    deg_in = np.maximum(np.bincount(dst, minlength=N_NODES), 1).astype(np.float32)

    node_core = graph_ids // GPC                      # node -> core
    src_half = (node_core[src] >= NCORES // 2).astype(np.int64)
    dA = np.bincount(dst[src_half == 0], minlength=N_NODES)
    dB = np.bincount(dst[src_half == 1], minlength=N_NODES)

    # pack nodes into groups per core
    core_nodes = [np.nonzero(node_core == c)[0] for c in range(NCORES)]
    packs = []
    Gmax = 0
    for c in range(NCORES):
        n = core_nodes[c]
        g, ng = _pack_groups(n, dA[n], dB[n])
        packs.append(g)
        Gmax = max(Gmax, ng)
    G = -(-Gmax // 4) * 4                             # multiple of 4 (supers of 4 groups)
    P = G // 2                                        # pairs (128-node tiles)
    NSUP = G // 4
    SH = G * GSZ                                      # rows per core shard
    R_half = (NCORES // 2) * SH
    assert R_half <= 32767, f"int16 overflow: {R_half}"

    # node -> row
    row = np.full(N_NODES, -1, dtype=np.int64)
    slot_in_group = np.zeros(N_NODES, dtype=np.int64)
    for c in range(NCORES):
        n = core_nodes[c]
        g = packs[c]
        order = np.lexsort((n, g))                    # stable by group
        n_sorted, g_sorted = n[order], g[order]
        # slot = rank within group
        slot = np.zeros(len(n), dtype=np.int64)
        _, starts = np.unique(g_sorted, return_index=True)
        for s0, s1 in zip(starts, list(starts[1:]) + [len(n)]):
            slot[s0:s1] = np.arange(s1 - s0)
        row[n_sorted] = c * SH + g_sorted * GSZ + slot
        slot_in_group[n_sorted] = slot

    # per-core edge layout
    e_core = node_core[dst]
    e_group = np.zeros(N_EDGES, dtype=np.int64)
    for c in range(NCORES):
        m = e_core == c
        d_local = dst[m]
        lr = row[d_local] - c * SH
        e_group[m] = lr // GSZ
    e_dslot = (row[dst] % SH) % GSZ
    e_srow = row[src] - src_half * R_half             # int16-safe source row

    per_core = []
    for c in range(NCORES):
        m = np.nonzero(e_core == c)[0]
        g, h, sr, dslt = e_group[m], src_half[m], e_srow[m], e_dslot[m]
        order = np.lexsort((sr, h, g))
        g, h, sr, dslt = g[order], h[order], sr[order], dslt[order]
        # rank within (g, h)
        key = g * 2 + h
        rank = np.arange(len(m)) - np.searchsorted(key, key, side="left")
        k = rank // 128                               # chunk within (g,h)
        p = rank % 128
        assert (k < CHUNKS_PER_HALF).all(), "cap exceeded"
        gi = g % 4                                    # group idx in super
        s = g // 4
        c16 = gi * CHUNKS_PER_HALF + k                # chunk col within (super, half)
        j = c16 * 128 + p                             # slot within (super, half)

        # idx arrays [2*NSUP, 16, 128] (then tiled to 128 partitions)
        idx16 = np.zeros((2 * NSUP, 16, 128), dtype=np.int16)
        t = s * 2 + h
        idx16[t, j % 16, j // 16] = sr.astype(np.int16)
        idx_all = np.tile(idx16, (1, 8, 1)).reshape(2 * NSUP, 128, 128)
        idx_2d = idx_all.transpose(1, 0, 2).reshape(128, 2 * NSUP * 128).copy()

        # dst one-hot scalars [128, 2*NSUP*16], -1 for pad slots
        dstv = np.full((128, 2 * NSUP * 16), -1.0, dtype=np.float32)
        dstv[j % 128, t * 16 + c16] = dslt.astype(np.float32)

        # per-pair node scalars [128, P]
        nodes_c = core_nodes[c]
        lr = row[nodes_c] - c * SH
        deg_in_t = np.ones((128, P), dtype=np.float32)
        deg_out_t = np.ones((128, P), dtype=np.float32)
        gid_t = np.full((128, P), -1.0, dtype=np.float32)
        pr = lr // 128
        pp = lr % 128
        deg_in_t[pp, pr] = deg_in[nodes_c]
        deg_out_t[pp, pr] = deg_out[nodes_c]
        gid_t[pp, pr] = (graph_ids[nodes_c] - c * GPC).astype(np.float32)

        counts = np.maximum(
            np.bincount(graph_ids[nodes_c] - c * GPC, minlength=GPC), 1
        ).astype(np.float32).reshape(GPC, 1)

        feats_shard = np.zeros((SH, D), dtype=np.float32)
        feats_shard[lr] = feats[nodes_c]

        per_core.append(dict(
            idx=idx_2d, dstv=dstv, deg_in=deg_in_t, deg_out=deg_out_t,
            gid=gid_t, counts=counts, feats=feats_shard,
        ))

    b_rep = np.broadcast_to(
        np.asarray(b, dtype=np.float32)[None, :, :], (128, N_LAYERS, D)
    ).copy()
    meta = dict(G=G, P=P, NSUP=NSUP, SH=SH, R_half=R_half)
    shared = dict(W=np.ascontiguousarray(np.asarray(W, dtype=np.float32).transpose(1, 0, 2)),
                  b_rep=b_rep,
                  scr=np.zeros((NCORES * SH, D), dtype=np.float32))
    return per_core, shared, meta


# ------------------------------------------------------------- device build
def build(meta, rep=1):
    G, P, NSUP, SH = meta["G"], meta["P"], meta["NSUP"], meta["SH"]
    R_half = meta["R_half"]
    CH = CHUNKS_PER_HALF
    f32 = mybir.dt.float32

    nc = bacc.Bacc("TRN2", target_bir_lowering=False, debug=False,
                   num_devices=NCORES)

    idx_t = nc.dram_tensor("idx", [128, 2 * NSUP * 128], mybir.dt.int16, kind="ExternalInput")
    dstv_t = nc.dram_tensor("dstv", [128, 2 * NSUP * 16], f32, kind="ExternalInput")
    degi_t = nc.dram_tensor("deg_in", [128, P], f32, kind="ExternalInput")
    dego_t = nc.dram_tensor("deg_out", [128, P], f32, kind="ExternalInput")
    gid_t = nc.dram_tensor("gid", [128, P], f32, kind="ExternalInput")
    cnt_t = nc.dram_tensor("counts", [GPC, 1], f32, kind="ExternalInput")
    feats_t = nc.dram_tensor("feats", [SH, D], f32, kind="ExternalInput")
    W_t = nc.dram_tensor("W", [128, N_LAYERS, D], f32, kind="ExternalInput")
    brep_t = nc.dram_tensor("b_rep", [128, N_LAYERS, D], f32, kind="ExternalInput")
    scr_t = nc.dram_tensor("scr", [NCORES * SH, D], f32, kind="ExternalInput")
    out_t = nc.dram_tensor("out", [GPC, D], f32, kind="ExternalOutput")

    hn_full = [
        nc.dram_tensor(f"hn_full{i}", [NCORES * SH, D], f32,
                       kind="Internal", addr_space="Shared")
        for i in range(N_LAYERS)
    ]

    with tile.TileContext(nc) as tc, ExitStack() as ctx:
        dram = ctx.enter_context(tc.tile_pool(name="dram", bufs=1, space="DRAM"))
        stat = ctx.enter_context(tc.tile_pool(name="stat", bufs=1))
        gpool = ctx.enter_context(tc.tile_pool(name="gath", bufs=4))
        opool = ctx.enter_context(tc.tile_pool(name="oh", bufs=4))
        spool = ctx.enter_context(tc.tile_pool(name="sb", bufs=4))
        ppool = ctx.enter_context(tc.tile_pool(name="agg_ps", bufs=3, space="PSUM"))
        hpool = ctx.enter_context(tc.tile_pool(name="h_ps", bufs=2, space="PSUM"))
        plpool = ctx.enter_context(tc.tile_pool(name="pool_ps", bufs=1, space="PSUM"))

        hn_shard = dram.tile([SH, D], f32)

        # ---- statics
        idx_sb = stat.tile([128, 2 * NSUP * 128], mybir.dt.int16)
        nc.sync.dma_start(idx_sb[:], idx_t.ap())
        dstv_sb = stat.tile([128, 2 * NSUP * 16], f32)
        nc.sync.dma_start(dstv_sb[:], dstv_t.ap())
        W_sb = stat.tile([128, N_LAYERS, D], f32)
        nc.sync.dma_start(W_sb[:], W_t.ap())
        brep_sb = stat.tile([128, N_LAYERS, D], f32)
        nc.sync.dma_start(brep_sb[:], brep_t.ap())
        gid_sb = stat.tile([128, P], f32)
        nc.sync.dma_start(gid_sb[:], gid_t.ap())
        cnt_sb = stat.tile([GPC, 1], f32)
        nc.sync.dma_start(cnt_sb[:], cnt_t.ap())

        degi_sb = stat.tile([128, P], f32)
        nc.sync.dma_start(degi_sb[:], degi_t.ap())
        dego_sb = stat.tile([128, P], f32)
        nc.sync.dma_start(dego_sb[:], dego_t.ap())
        ni_sb = stat.tile([128, P], f32)   # rsqrt(deg_in)
        no_sb = stat.tile([128, P], f32)   # rsqrt(deg_out)
        nc.vector.reciprocal(ni_sb[:], degi_sb[:])
        nc.scalar.activation(ni_sb[:], ni_sb[:], mybir.ActivationFunctionType.Sqrt)
        nc.vector.reciprocal(no_sb[:], dego_sb[:])
        nc.scalar.activation(no_sb[:], no_sb[:], mybir.ActivationFunctionType.Sqrt)
        rc_sb = stat.tile([GPC, 1], f32)   # 1/counts
        nc.vector.reciprocal(rc_sb[:], cnt_sb[:])

        iota16 = stat.tile([128, GSZ], mybir.dt.int16)
        nc.gpsimd.iota(iota16[:], pattern=[[1, GSZ]], base=0, channel_multiplier=0)
        iota_f = stat.tile([128, GSZ], f32)
        nc.vector.tensor_copy(iota_f[:], iota16[:])

        # graph one-hot [128, P, GPC] (built once; pooling uses layer-2 h)
        groh = stat.tile([128, P, GPC], f32)
        nc.vector.tensor_tensor(
            out=groh[:],
            in0=iota_f[:, :GPC].unsqueeze(1).broadcast_to([128, P, GPC]),
            in1=gid_sb[:].unsqueeze(2).broadcast_to([128, P, GPC]),
            op=mybir.AluOpType.is_equal,
        )

        for _ in range(rep):
            # ---- layer 0 input: hn0 = feats * norm_out
            for pr in range(P):
                ft = spool.tile([128, D], f32, tag="ft")
                nc.sync.dma_start(ft[:], feats_t.ap()[pr * 128:(pr + 1) * 128, :])
                hn0 = spool.tile([128, D], f32, tag="hn")
                nc.vector.tensor_scalar_mul(hn0[:], ft[:], no_sb[:, pr:pr + 1])
                nc.sync.dma_start(hn_shard[pr * 128:(pr + 1) * 128, :], hn0[:])
            nc.gpsimd.collective_compute(
                "AllGather", mybir.AluOpType.bypass,
                replica_groups=[list(range(NCORES))],
                ins=[hn_shard.opt()], outs=[hn_full[0].ap().opt()],
            )
            nc.sync.dma_start(scr_t.ap(), hn_full[0].ap())

            pool_ps = plpool.tile([GPC, D], f32)

            for l in range(N_LAYERS):
                for s in range(NSUP):
                    gA = gpool.tile([128, 4 * CH, D], f32, tag="gA")
                    gB = gpool.tile([128, 4 * CH, D], f32, tag="gB")
                    nc.gpsimd.dma_gather(
                        out_ap=gA[:], in_ap=scr_t.ap()[0:R_half, :],
                        idxs_ap=idx_sb[:, (2 * s) * 128:(2 * s + 1) * 128],
                        num_idxs=4 * CH * 128, num_idxs_reg=4 * CH * 128,
                        elem_size=D, single_packet=False,
                    )
                    nc.gpsimd.dma_gather(
                        out_ap=gB[:], in_ap=scr_t.ap()[R_half:, :],
                        idxs_ap=idx_sb[:, (2 * s + 1) * 128:(2 * s + 2) * 128],
                        num_idxs=4 * CH * 128, num_idxs_reg=4 * CH * 128,
                        elem_size=D, single_packet=False,
                    )
                    ohA = opool.tile([128, 4 * CH, GSZ], f32, tag="ohA")
                    ohB = opool.tile([128, 4 * CH, GSZ], f32, tag="ohB")
                    nc.vector.tensor_tensor(
                        out=ohA[:],
                        in0=iota_f[:].unsqueeze(1).broadcast_to([128, 4 * CH, GSZ]),
                        in1=dstv_sb[:, (2 * s) * 16:(2 * s) * 16 + 16]
                            .unsqueeze(2).broadcast_to([128, 4 * CH, GSZ]),
                        op=mybir.AluOpType.is_equal,
                    )
                    nc.vector.tensor_tensor(
                        out=ohB[:],
                        in0=iota_f[:].unsqueeze(1).broadcast_to([128, 4 * CH, GSZ]),
                        in1=dstv_sb[:, (2 * s + 1) * 16:(2 * s + 1) * 16 + 16]
                            .unsqueeze(2).broadcast_to([128, 4 * CH, GSZ]),
                        op=mybir.AluOpType.is_equal,
                    )
                    for pi in range(2):         # pairs in super
                        pr = s * 2 + pi
                        agg = ppool.tile([128, 128], f32, tag="agg")
                        for gj in range(2):     # groups in pair
                            gi = pi * 2 + gj
                            off = gj * GSZ
                            for k in range(CH):
                                nc.tensor.matmul(
                                    agg[:, off:off + GSZ],
                                    gA[:, gi * CH + k, :],
                                    ohA[:, gi * CH + k, :],
                                    start=(k == 0), stop=False,
                                    skip_group_check=True,
                                )
                            for k in range(CH):
                                nc.tensor.matmul(
                                    agg[:, off:off + GSZ],
                                    gB[:, gi * CH + k, :],
                                    ohB[:, gi * CH + k, :],
                                    start=False, stop=(k == CH - 1),
                                    skip_group_check=True,
                                )
                        agg_sb = spool.tile([128, 128], f32, tag="aggsb")
                        nc.scalar.copy(agg_sb[:], agg[:])
                        hps = hpool.tile([128, D], f32, tag="hps")
                        nc.tensor.matmul(hps[:], agg_sb[:], W_sb[:, l, :],
                                         start=True, stop=True)
                        t_sb = spool.tile([128, D], f32, tag="tsb")
                        nc.vector.scalar_tensor_tensor(
                            out=t_sb[:], in0=hps[:], scalar=ni_sb[:, pr:pr + 1],
                            in1=brep_sb[:, l, :],
                            op0=mybir.AluOpType.mult, op1=mybir.AluOpType.add,
                        )
                        if l < N_LAYERS - 1:
                            hn = spool.tile([128, D], f32, tag="hn2")
                            nc.vector.tensor_scalar(
                                out=hn[:], in0=t_sb[:],
                                scalar1=0.0, scalar2=no_sb[:, pr:pr + 1],
                                op0=mybir.AluOpType.max, op1=mybir.AluOpType.mult,
                            )
                            nc.sync.dma_start(
                                hn_shard[pr * 128:(pr + 1) * 128, :], hn[:])
                        else:
                            h_sb = spool.tile([128, D], f32, tag="hsb")
                            nc.vector.tensor_scalar_max(h_sb[:], t_sb[:], 0.0)
                            nc.tensor.matmul(
                                pool_ps[:], groh[:, pr, :], h_sb[:],
                                start=(pr == 0), stop=(pr == P - 1),
                            )
                if l < N_LAYERS - 1:
                    nc.gpsimd.collective_compute(
                        "AllGather", mybir.AluOpType.bypass,
                        replica_groups=[list(range(NCORES))],
                        ins=[hn_shard.opt()], outs=[hn_full[l + 1].ap().opt()],
                    )
                    nc.sync.dma_start(scr_t.ap(), hn_full[l + 1].ap())

            pool_sb = spool.tile([GPC, D], f32, tag="poolsb")
            nc.vector.tensor_scalar_mul(pool_sb[:], pool_ps[:], rc_sb[:])
            nc.sync.dma_start(out_t.ap(), pool_sb[:])

    nc.compile()
    return nc


def make_in_maps(per_core, shared):
    in_maps = []
    for c in range(NCORES):
        pc = per_core[c]
        in_maps.append({
            "idx": pc["idx"], "dstv": pc["dstv"], "deg_in": pc["deg_in"],
            "deg_out": pc["deg_out"], "gid": pc["gid"], "counts": pc["counts"],
            "feats": pc["feats"], "W": shared["W"], "b_rep": shared["b_rep"],
            "scr": shared["scr"],
        })
    return in_maps


def kernel(**inputs) -> np.ndarray:
    per_core, shared, meta = preprocess(**inputs)
    nc = build(meta, rep=1)
    in_maps = make_in_maps(per_core, shared)
    res = run_bass_kernel_spmd(nc, in_maps, core_ids=list(range(NCORES)))
    return np.concatenate([res.results[c]["out"] for c in range(NCORES)], axis=0)
